# revision 1
# baseline (speedup 1.0000x reference)
"""Trainium2 Bass kernel for nn_CompositeEmbeddingA (octree composite embedding).

Per sample (1 sample per NeuronCore, batch=8 over 8 cores):
  layers 0-2 (depths 1-3): x = val_emb[v] + pos0[p0] + pos1[p1] + pos2[p2] + dep_emb[d]
  layers 3-4: same sum w/o dep, then Conv1d(E,E,kernel=stride=k), k=4 (l3) / 8 (l4)

Algorithm: every layer is expressed as  out = MultiHot @ Table  on the PE:
  - conv folded into the tables host-side: per tap j, T_j = concat(tables) @ w[:,:,j].T,
    so out[t] = sum_j multihot(token 8t+j) @ T_j  == one K=(196k) matmul per layer.
  - MultiHot^T (contraction dim on partitions) is built on-chip:
      PE "broadcast matmul": bcast[r_row, tok] = selector^T @ idx_rows  (replicates the
      right index value into every table row), then DVE is_equal against a per-partition
      constant column -> exact 0/1 one-hot, fp32.
  - conv bias = one extra table row whose selector column is all-zero (bcast value 0)
    with compare const 0 -> fires for every token.
  - main matmuls run in float32r (full fp32 data, 1 cycle/row at N>=256).
"""

import sys

for _p in ("/opt/trn_rl_repo",):
    if _p not in sys.path:
        sys.path.insert(0, _p)

import numpy as np
import ml_dtypes

RES = 32
SPATIAL = 3
NUM_VOCAB = 3
E = 256
BATCH = 8
LAYER_SIZES = (8, 64, 512, 4096, 32768)
CONV_SIZE = {3: 4, 4: 8}
S_TOTAL = sum(LAYER_SIZES)  # 37448
OUT_TOKENS = 8 + 64 + 512 + 1024 + 4096  # 5704
NIDX = 33  # 32 idx rows + one all-ones row (carries the -c compare constants)
ONES_ROW = 32
STRIPE = 512

# segment widths inside one tap: value(4), pos0(64), pos1(64), pos2(64) [, dep(6)]
SEG_W = (NUM_VOCAB + 1, 2 * RES, 2 * RES, 2 * RES)
DEP_W = 6

_BF16 = ml_dtypes.bfloat16


def _layer_slices():
    out = []
    start = 0
    for n in LAYER_SIZES:
        out.append((start, start + n))
        start += n
    return out


LAYER_SL = _layer_slices()


def _build_consts(params):
    """Fold conv weights into tables; pack rows into 128-row chunks.

    Returns (tbl [NC,128,256] f32, sel [NC,32,128] bf16, cval [NC,128,1] f32,
             layers: list of (name, T_tokens, out_offset, chunk_index_list))
    """
    rows_tbl = []   # per logical row: the 256-vector
    rows_ridx = []  # which of the 32 idx rows feeds this row (-1 = none: bcast val 0)
    rows_c = []     # compare constant
    layer_marks = []  # (row_start, row_end) per virtual layer

    def seg_tables(l):
        t = [np.asarray(params[f"val_emb_{l}"], np.float32)]
        pe = np.asarray(params[f"pos_emb_{l}"], np.float32)
        t += [pe[0], pe[1], pe[2]]
        return t

    # virtual layer "B": real layers 0..2 merged. idx rows: l*5 + (v,p0,p1,p2,d)
    r0 = len(rows_tbl)
    for l in range(3):
        tabs = seg_tables(l) + [np.asarray(params[f"dep_emb_{l}"], np.float32)]
        for seg, tab in enumerate(tabs):
            for c in range(tab.shape[0]):
                rows_tbl.append(tab[c])
                rows_ridx.append(l * 5 + seg)
                rows_c.append(float(c))
    layer_marks.append((r0, len(rows_tbl)))

    # conv layers: idx rows j*4+seg; one bias row (all-zero selector col, c=0)
    for l in (3, 4):
        r0 = len(rows_tbl)
        k = CONV_SIZE[l]
        w = np.asarray(params[f"conv_w_{l}"], np.float32)  # [O, E, k]
        b = np.asarray(params[f"conv_b_{l}"], np.float32)  # [O]
        tabs = seg_tables(l)
        for j in range(k):
            wj = w[:, :, j]  # [O, E]
            for seg, tab in enumerate(tabs):
                folded = tab @ wj.T  # [rows, O]
                for c in range(tab.shape[0]):
                    rows_tbl.append(folded[c])
                    rows_ridx.append(j * 4 + seg)
                    rows_c.append(float(c))
        rows_tbl.append(b)
        rows_ridx.append(-1)
        rows_c.append(0.0)
        layer_marks.append((r0, len(rows_tbl)))

    # chunkify each virtual layer into 128-row chunks
    tbl_chunks, sel_chunks, cval_chunks = [], [], []
    layers = []
    out_offs = [0, 584, 1608]
    names = ["B", "L3", "L4"]
    t_counts = [584, 1024, 4096]
    for vl, (r0, r1) in enumerate(layer_marks):
        n = r1 - r0
        nch = -(-n // 128)
        cids = []
        for ci in range(nch):
            a = r0 + ci * 128
            bnd = min(r0 + (ci + 1) * 128, r1)
            rows = bnd - a
            tbl = np.zeros((128, E), np.float32)
            sel = np.zeros((NIDX, 128), np.float32)
            sel[ONES_ROW, :] = 1.0  # pad rows: bcast value = +1 -> eq(.,0)=0
            for m in range(rows):
                tbl[m] = rows_tbl[a + m]
                if rows_ridx[a + m] >= 0:
                    sel[rows_ridx[a + m], m] = 1.0
                # ones-row coefficient: broadcast out = idx - c
                sel[ONES_ROW, m] = -rows_c[a + m]
            cids.append(len(tbl_chunks))
            tbl_chunks.append(tbl)
            sel_chunks.append(sel.astype(_BF16))
        layers.append((names[vl], t_counts[vl], out_offs[vl], cids))

    # merged layouts: one DMA per constant tensor
    tbl = np.concatenate(tbl_chunks, axis=1)  # [128, NC*256] f32
    sel = np.concatenate(sel_chunks, axis=1)  # [33, NC*128] bf16
    return tbl, sel, layers


def _build_ridx(value, depth, position, b):
    """Per-core index-row tensors, one per virtual layer: [32, T] bf16."""
    out = {}
    # B: merged layers 0-2; out tokens 0..583 = input tokens 0..583
    rb = np.full((NIDX, 584), -1.0, np.float32)
    rb[ONES_ROW] = 1.0
    col = 0
    for l in range(3):
        lo, hi = LAYER_SL[l]
        n = hi - lo
        rb[l * 5 + 0, col : col + n] = value[b, lo:hi]
        for s in range(3):
            rb[l * 5 + 1 + s, col : col + n] = position[b, lo:hi, s]
        rb[l * 5 + 4, col : col + n] = depth[b, lo:hi]
        col += n
    out["B"] = rb.astype(_BF16)
    for name, l in (("L3", 3), ("L4", 4)):
        k = CONV_SIZE[l]
        lo, hi = LAYER_SL[l]
        T = (hi - lo) // k
        r = np.zeros((NIDX, T), np.float32)
        r[ONES_ROW] = 1.0
        for j in range(k):
            r[j * 4 + 0] = value[b, lo:hi][j::k]
            for s in range(3):
                r[j * 4 + 1 + s] = position[b, lo:hi, s][j::k]
        out[name] = r.astype(_BF16)
    return out


_CACHE = {}

# schedule tuning knobs (sweepable via analyze_sweep.py)
PAIR = 1  # chunks fused per eq op
BPS_BUFS = 5
OPS_BUFS = 3
MH_BUFS = 3
ACT_MOD = 4  # pair p goes to ACT when p % ACT_MOD == ACT_MOD - 1
DEPTH = 2
STAGE = "full"  # "full" | "mh_only" | "main_only" (HW bisection)
EQ_BF16 = False  # bf16 PSUM matmul output is TRN3-only
TT_PAIR = 1  # main t-tiles packed per PSUM bank (2 regressed on HW: 311us)


def _get_nc(layers, nchunks, reps=1):
    key = ("v1", PAIR, BPS_BUFS, OPS_BUFS, MH_BUFS, ACT_MOD, DEPTH, reps, STAGE,
           EQ_BF16, TT_PAIR, tuple((n, t, o, tuple(c)) for n, t, o, c in layers))
    if key in _CACHE:
        return _CACHE[key]

    import concourse.bass as bass
    import concourse.tile as tile
    from concourse import bacc, mybir
    from contextlib import ExitStack

    f32 = mybir.dt.float32
    f32r = mybir.dt.float32r
    bf16 = mybir.dt.bfloat16

    nc = bacc.Bacc(trn_type="TRN2", target_bir_lowering=False, debug=False)
    tbl_d = nc.dram_tensor("tbl", [128, nchunks * E], f32r, kind="ExternalInput").ap()
    sel_d = nc.dram_tensor(
        "sel", [NIDX, nchunks * 128], bf16, kind="ExternalInput"
    ).ap()
    ridx_d = {
        name: nc.dram_tensor(f"ridx_{name}", [NIDX, T], bf16, kind="ExternalInput").ap()
        for name, T, _, _ in layers
    }
    out_d = nc.dram_tensor("out", [OUT_TOKENS, E], f32, kind="ExternalOutput").ap()

    with tile.TileContext(nc) as tc, ExitStack() as ctx:
        cpool = ctx.enter_context(tc.tile_pool(name="const", bufs=1))
        rpool = ctx.enter_context(tc.tile_pool(name="ridx", bufs=DEPTH + 1))
        mpool = ctx.enter_context(tc.tile_pool(name="mh", bufs=MH_BUFS))
        tpool = ctx.enter_context(tc.tile_pool(name="sq", bufs=3))
        bps = ctx.enter_context(
            tc.tile_pool(name="bps", bufs=BPS_BUFS, space=bass.MemorySpace.PSUM)
        )
        ops = ctx.enter_context(
            tc.tile_pool(name="ops", bufs=OPS_BUFS, space=bass.MemorySpace.PSUM)
        )
        opool = ctx.enter_context(tc.tile_pool(name="osb", bufs=3))

        # small consts first so the first broadcast matmuls start immediately;
        # the big table load is split per-layer in use order behind them
        sel_t = cpool.tile([NIDX, nchunks * 128], bf16, tag="sel")
        nc.sync.dma_start(sel_t[:], sel_d[:])
        tbl_t = cpool.tile([128, nchunks * E], f32r, tag="tbl")
        for _, _, _, cids in layers:
            lo, hi = cids[0] * E, (cids[-1] + 1) * E
            nc.sync.dma_start(tbl_t[:, lo:hi], tbl_d[:, lo:hi])

        A = mybir.ActivationFunctionType
        stripes = []
        for name, T, out_off, cids in layers:
            for s0 in range(0, T, STRIPE):
                stripes.append((name, out_off, cids, s0, min(STRIPE, T - s0)))
        # spread the small eq-heavy stripes (B/L3) between PE-heavy L4 ones
        big = [s for s in stripes if s[0] == "L4"]
        small = [s for s in stripes if s[0] != "L4"]
        small.sort(key=lambda s: -s[4])  # tiny tail stripe goes last
        stripes = []
        for i, b in enumerate(big):
            stripes.append(b)
            if i * len(small) // len(big) < (i + 1) * len(small) // len(big):
                stripes.append(small[i * len(small) // len(big)])

        def load_ridx(si):
            name, _, cids, s0, W = stripes[si]
            rt = rpool.tile([NIDX, W], bf16, tag="r")
            nc.sync.dma_start(rt[:], ridx_d[name][:, s0 : s0 + W])
            return rt

        def build_mh_pair(si, rt, p, ks):
            """broadcast matmuls + eq for a pair (or single) of chunks.

            The broadcast output is already idx - c (ones-row trick), so the
            one-hot is a compare against immediate 0 and one DVE/ACT op can
            span both chunks of the pair.
            """
            _, _, cids, _, W = stripes[si]
            n = len(ks)
            bp = bps.tile([128, n * W], bf16 if EQ_BF16 else f32, tag="b")
            for i, k in enumerate(ks):
                ci = cids[k]
                nc.tensor.matmul(
                    bp[:, i * W : (i + 1) * W],
                    sel_t[:, ci * 128 : (ci + 1) * 128],
                    rt[:],
                    start=True,
                    stop=True,
                )
            mh = mpool.tile([128, n * W], f32r, tag=f"mh{p}")
            if p % ACT_MOD == ACT_MOD - 1:
                # ACT path: relu(1 - x^2) — exact 0/1 for integer x
                tmp = tpool.tile([128, n * W], f32, tag="sq")
                nc.scalar.activation(tmp[:], bp[:], A.Square)
                nc.scalar.activation(mh[:], tmp[:], A.Relu, bias=1.0, scale=-1.0)
            else:
                nc.vector.tensor_scalar(
                    mh[:], bp[:], 0.0, None, op0=mybir.AluOpType.is_equal
                )
            return [mh[:, i * W : (i + 1) * W] for i in range(n)]

        def main_ttile(si, mhs, ti, ob):
            """two t-tiles packed into one PSUM bank; one evict per pair."""
            _, _, cids, _, W = stripes[si]
            nt = min(TT_PAIR, -(-W // 128) - TT_PAIR * ti)
            op = ops.tile([128, nt * E], f32, tag="o")
            Ms = []
            for h in range(nt):
                t0 = (TT_PAIR * ti + h) * 128
                M = min(128, W - t0)
                Ms.append(M)
                for k, ci in enumerate(cids):
                    nc.tensor.matmul(
                        op[:M, h * E : h * E + E],
                        mhs[k][:, t0 : t0 + M],
                        tbl_t[:, ci * E : (ci + 1) * E],
                        start=(k == 0),
                        stop=(k == len(cids) - 1),
                    )
            col = TT_PAIR * ti * E
            if nt == 2 and Ms[0] == 128 and Ms[1] == 128:
                nc.scalar.activation(ob[:, col : col + 2 * E], op[:], A.Copy)
            else:
                for h in range(nt):
                    nc.scalar.activation(
                        ob[: Ms[h], col + h * E : col + (h + 1) * E],
                        op[: Ms[h], h * E : h * E + E],
                        A.Copy,
                    )

        def store_out(si, ob):
            _, out_off, _, s0, W = stripes[si]
            row = out_off + s0
            if W % 128 == 0:
                dst = out_d[row : row + W, :].rearrange("(a p) e -> p a e", p=128)
                src = ob[:].rearrange("p (a e) -> p a e", e=E)
                nc.sync.dma_start(dst, src)
            else:
                nc.sync.dma_start(out_d[row : row + W, :], ob[:W, :E])

        # two-stripe software pipeline with interleaved emission: pair-builds
        # of stripe s+2's one-hots alternate with stripe s's main t-tiles.
        def stripe_pairs(si):
            nk = len(stripes[si][2])
            return [tuple(range(a, min(a + PAIR, nk))) for a in range(0, nk, PAIR)]

        def emit_pairs(si, rt, prs):
            mhs = []
            for p, ks in prs:
                mhs += build_mh_pair(si, rt, p, ks)
            return mhs

        def emit_body_mh_only():
            for si in range(len(stripes)):
                rt = load_ridx(si)
                emit_pairs(si, rt, list(enumerate(stripe_pairs(si))))

        static_mh = {}
        if STAGE == "main_only":
            tmp0 = cpool.tile([128, STRIPE], f32, tag="smhtmp")
            nc.gpsimd.memset(tmp0[:], 0.5)
            for p in range(13):
                t = cpool.tile([128, STRIPE], f32r, tag=f"smh{p}")
                nc.vector.tensor_scalar(
                    t[:], tmp0[:], 0.0, None, op0=mybir.AluOpType.is_equal
                )
                static_mh[p] = t

        def emit_body_main_only():
            for si in range(len(stripes)):
                _, _, cids, _, W = stripes[si]
                ntt = -(-W // 128)
                ob = opool.tile([128, ntt * E], f32, tag="ob")
                mhs = [static_mh[k][:, :W] for k in range(len(cids))]
                for ti in range(-(-ntt // TT_PAIR)):
                    main_ttile(si, mhs, ti, ob)
                store_out(si, ob)

        def emit_body():
            nst = len(stripes)
            mh_of = {}
            for si in range(min(DEPTH, nst)):
                rt = load_ridx(si)
                mh_of[si] = emit_pairs(si, rt, list(enumerate(stripe_pairs(si))))
            for si in range(nst):
                W = stripes[si][4]
                ntt = -(-W // 128)
                ngr = -(-ntt // TT_PAIR)
                ob = opool.tile([128, ntt * E], f32, tag="ob")
                sj = si + DEPTH
                if sj < nst:
                    rt = load_ridx(sj)
                    prs = list(enumerate(stripe_pairs(sj)))
                    npr = len(prs)
                    # split stripe sj's pair-builds into groups interleaved
                    # with stripe si's main t-tile pairs
                    bounds = [round(g * npr / ngr) for g in range(ngr + 1)]
                    mh_of[sj] = []
                    for ti in range(ngr):
                        main_ttile(si, mh_of[si], ti, ob)
                        mh_of[sj] += emit_pairs(
                            sj, rt, prs[bounds[ti] : bounds[ti + 1]]
                        )
                else:
                    for ti in range(ngr):
                        main_ttile(si, mh_of[si], ti, ob)
                store_out(si, ob)
                del mh_of[si]

        body_fn = {
            "full": emit_body,
            "mh_only": emit_body_mh_only,
            "main_only": emit_body_main_only,
        }[STAGE]
        if reps == 1:
            body_fn()
        else:
            # timing mode: repeat the body on-device to measure per-iter HW
            # time as a wall-clock slope (no NTFF profiling available)
            hints = (
                mybir.EngineType.PE,
                mybir.EngineType.DVE,
                mybir.EngineType.Activation,
                mybir.EngineType.SP,
            )
            with tc.For_i(0, reps, 1, hint_engines=hints):
                body_fn()

    nc.compile()
    _CACHE[key] = nc
    return nc


def kernel(**inputs):
    from concourse.bass_utils import run_bass_kernel_spmd

    value = np.asarray(inputs["value"], np.int32).astype(np.float32)
    depth = np.asarray(inputs["depth"], np.int32).astype(np.float32)
    position = np.asarray(inputs["position"], np.int32).astype(np.float32)

    tbl, sel, layers = _build_consts(inputs)
    nc = _get_nc(layers, tbl.shape[1] // E)

    in_maps = []
    for b in range(BATCH):
        rid = _build_ridx(value, depth, position, b)
        m = {"tbl": tbl, "sel": sel}
        for name, _, _, _ in layers:
            m[f"ridx_{name}"] = rid[name]
        in_maps.append(m)

    res = run_bass_kernel_spmd(nc, in_maps, list(range(BATCH)))
    return np.stack([res.results[b]["out"] for b in range(BATCH)])



# revision 43
# speedup vs baseline: 1.7488x; 1.7488x over previous
"""Trainium2 Bass kernel for nn_CompositeEmbeddingA (octree composite embedding).

Per sample (1 sample per NeuronCore, batch=8 over 8 cores):
  layers 0-2 (depths 1-3): x = val_emb[v] + pos0[p0] + pos1[p1] + pos2[p2] + dep_emb[d]
  layers 3-4: same sum w/o dep, then Conv1d(E,E,kernel=stride=k), k=4 (l3) / 8 (l4)

Algorithm: every layer is expressed as  out = MultiHot @ Table  on the PE:
  - conv folded into the tables host-side: per tap j, T_j = concat(tables) @ w[:,:,j].T,
    so out[t] = sum_j multihot(token 8t+j) @ T_j  == one K=(196k) matmul per layer.
  - MultiHot^T (contraction dim on partitions) is built on-chip:
      PE "broadcast matmul": bcast[r_row, tok] = selector^T @ idx_rows  (replicates the
      right index value into every table row), then DVE is_equal against a per-partition
      constant column -> exact 0/1 one-hot, fp32.
  - conv bias = one extra table row whose selector column is all-zero (bcast value 0)
    with compare const 0 -> fires for every token.
  - main matmuls run in float32r (full fp32 data, 1 cycle/row at N>=256).
"""

import sys

for _p in ("/opt/trn_rl_repo",):
    if _p not in sys.path:
        sys.path.insert(0, _p)

import numpy as np
import ml_dtypes

RES = 32
SPATIAL = 3
NUM_VOCAB = 3
E = 256
BATCH = 8
LAYER_SIZES = (8, 64, 512, 4096, 32768)
CONV_SIZE = {3: 4, 4: 8}
S_TOTAL = sum(LAYER_SIZES)  # 37448
OUT_TOKENS = 8 + 64 + 512 + 1024 + 4096  # 5704
NIDX = 33  # 32 idx rows + one all-ones row (carries the -c compare constants)
ONES_ROW = 32
# d^2-form selector row layout: idx rows, idx^2 split rows, two ones rows
# (c^2 split across two all-ones rows to stay bf16-exact for c up to 63)
SQHI = 33
SQLO = 65
ONES_A = 97
ONES_B = 98
NIDXQ = 99
STRIPE = 512

# segment widths inside one tap: value(4), pos0(64), pos1(64), pos2(64) [, dep(6)]
SEG_W = (NUM_VOCAB + 1, 2 * RES, 2 * RES, 2 * RES)
DEP_W = 6

_BF16 = ml_dtypes.bfloat16
_E4 = ml_dtypes.float8_e4m3fn if hasattr(ml_dtypes, "float8_e4m3fn") else ml_dtypes.float8_e4m3


def _layer_slices():
    out = []
    start = 0
    for n in LAYER_SIZES:
        out.append((start, start + n))
        start += n
    return out


LAYER_SL = _layer_slices()


def _build_consts(params):
    """Fold conv weights into tables; pack rows into 128-row chunks.

    Returns (tbl [NC,128,256] f32, sel [NC,32,128] bf16, cval [NC,128,1] f32,
             layers: list of (name, T_tokens, out_offset, chunk_index_list))
    """
    rows_tbl = []   # per logical row: the 256-vector
    rows_ridx = []  # which of the 32 idx rows feeds this row (-1 = none: bcast val 0)
    rows_c = []     # compare constant
    layer_marks = []  # (row_start, row_end) per virtual layer

    def seg_tables(l):
        t = [np.asarray(params[f"val_emb_{l}"], np.float32)]
        pe = np.asarray(params[f"pos_emb_{l}"], np.float32)
        t += [pe[0], pe[1], pe[2]]
        return t

    # per-depth basic layers B0..B2 (202 rows each -> 2 chunks, so a token
    # only contracts its own depth's chunks). idx rows: seg (v,p0,p1,p2,d)
    for l in range(3):
        r0 = len(rows_tbl)
        tabs = seg_tables(l) + [np.asarray(params[f"dep_emb_{l}"], np.float32)]
        for seg, tab in enumerate(tabs):
            for c in range(tab.shape[0]):
                rows_tbl.append(tab[c])
                rows_ridx.append(seg)
                rows_c.append(float(c))
        layer_marks.append((r0, len(rows_tbl)))

    # conv layers: idx rows j*4+seg; one bias row (all-zero selector col, c=0)
    for l in (3, 4):
        r0 = len(rows_tbl)
        k = CONV_SIZE[l]
        w = np.asarray(params[f"conv_w_{l}"], np.float32)  # [O, E, k]
        b = np.asarray(params[f"conv_b_{l}"], np.float32)  # [O]
        tabs = seg_tables(l)
        for j in range(k):
            wj = w[:, :, j]  # [O, E]
            for seg, tab in enumerate(tabs):
                folded = tab @ wj.T  # [rows, O]
                for c in range(tab.shape[0]):
                    rows_tbl.append(folded[c])
                    rows_ridx.append(j * 4 + seg)
                    rows_c.append(float(c))
        rows_tbl.append(b)
        rows_ridx.append(-1)
        rows_c.append(0.0)
        layer_marks.append((r0, len(rows_tbl)))

    # chunkify each virtual layer into 128-row chunks. Two selector forms per
    # chunk (schedule picks per instance which engine builds the one-hot):
    #  - sel8 [NIDX, 2, 128] fp8, DoubleRow bcast producing d = 16*(h-ch)+(l-cl)
    #    with idx = 8h+l split into fp8-exact nibbles; eq on DVE (is_equal 0).
    #  - selq [NIDXQ, 128] bf16 bcast producing d^2 = idx^2 - 2c*idx + c^2
    #    exactly (idx^2 and c^2 split hi/lo to stay bf16-exact); one-hot on
    #    ACT in a single exact pass: relu(1 - d^2).
    tbl_chunks, sel8_chunks, selq_chunks = [], [], []
    layers = []
    out_offs = [0, 8, 72, 584, 1608]
    names = ["B0", "B1", "B2", "L3", "L4"]
    t_counts = [8, 64, 512, 1024, 4096]
    for vl, (r0, r1) in enumerate(layer_marks):
        n = r1 - r0
        nch = -(-n // 128)
        cids = []
        for ci in range(nch):
            a = r0 + ci * 128
            bnd = min(r0 + (ci + 1) * 128, r1)
            rows = bnd - a
            tbl = np.zeros((128, E), np.float32)
            sel8 = np.zeros((NIDX, 2, 128), np.float32)
            sel8[ONES_ROW, 1, :] = 1.0  # pad rows: d = +1 -> no fire
            selq = np.zeros((NIDXQ, 128), np.float32)
            selq[ONES_B, :] = 1.0  # pad rows: d^2 = 1 -> relu(1-1) = 0
            for m in range(rows):
                tbl[m] = rows_tbl[a + m]
                r = rows_ridx[a + m]
                c = int(rows_c[a + m])
                if r >= 0:
                    ch, cl = c // 8, c % 8
                    sel8[r, 0, m] = 16.0
                    sel8[r, 1, m] = 1.0
                    sel8[ONES_ROW, 0, m] = -16.0 * ch
                    sel8[ONES_ROW, 1, m] = -float(cl)
                    selq[SQHI + r, m] = 1.0
                    selq[SQLO + r, m] = 1.0
                    selq[r, m] = -2.0 * c
                    selq[ONES_A, m] = float((c * c) // 64 * 64)
                    selq[ONES_B, m] = float((c * c) % 64)
                else:  # bias row: always fire
                    sel8[ONES_ROW, :, m] = 0.0
                    selq[ONES_A, m] = 0.0
                    selq[ONES_B, m] = 0.0
            cids.append(len(tbl_chunks))
            tbl_chunks.append(tbl)
            sel8_chunks.append(sel8.astype(_E4))
            selq_chunks.append(selq.astype(_BF16))
        layers.append((names[vl], t_counts[vl], out_offs[vl], cids))

    # merged layouts: one DMA per constant tensor.
    # Tables ship as fp8e4m3 (hi, lo) pairs interleaved per chunk: the main
    # matmuls run in DoubleRow perf mode (0.5 cycles/row) contracting both
    # k-tiles at once; hi+lo reconstructs fp32 to ~0.4% per element. A global
    # power-of-two scale puts values in fp8 range; evict descales by 1/s.
    tblf = np.concatenate(tbl_chunks, axis=1)  # [128, NC*256] f32
    sel8 = np.concatenate(
        [s.reshape(NIDX, 256) for s in sel8_chunks], axis=1
    )  # [33, NC*256] fp8
    selq = np.concatenate(selq_chunks, axis=1)  # [98, NC*128] bf16
    s = 2.0 ** np.floor(np.log2(240.0 / max(np.abs(tblf).max(), 1e-9)))
    hi = (tblf * s).astype(_E4).astype(np.float32)
    lo = (tblf * s - hi).astype(_E4)
    nch = tblf.shape[1] // E
    t4 = np.zeros((128, nch, 2, E), _E4)
    t4[:, :, 0, :] = hi.astype(_E4).reshape(128, nch, E)
    t4[:, :, 1, :] = lo.reshape(128, nch, E)
    return t4.reshape(128, nch * 2 * E), float(s), sel8, selq, layers


def _pack_ridx(idx):
    """idx [nrows, T] int -> (r8 [NIDX,2,T] fp8 nibbles+ones, rq [NIDXQ,T] bf16
    with idx, idx^2 hi/lo, and ones rows)."""
    nrows, T = idx.shape
    r8 = np.zeros((NIDX, 2, T), np.float32)
    r8[ONES_ROW, :, :] = 1.0
    r8[:nrows, 0, :] = idx // 8
    r8[:nrows, 1, :] = idx % 8
    rq = np.zeros((NIDXQ, T), np.float32)
    rq[ONES_A] = 1.0
    rq[ONES_B] = 1.0
    sq = idx * idx
    rq[:nrows] = idx
    rq[SQHI : SQHI + nrows] = (sq // 64) * 64
    rq[SQLO : SQLO + nrows] = sq % 64
    return r8.astype(_E4), rq.astype(_BF16)


def _build_ridx(value, depth, position, b):
    """Per-core per-layer index tensors in both selector forms."""
    out = {}
    for l in range(3):
        lo, hi = LAYER_SL[l]
        idx = np.stack(
            [value[b, lo:hi]]
            + [position[b, lo:hi, s] for s in range(3)]
            + [depth[b, lo:hi]]
        ).astype(np.int64)
        out[f"r8_B{l}"], out[f"rq_B{l}"] = _pack_ridx(idx)
    for name, l in (("L3", 3), ("L4", 4)):
        k = CONV_SIZE[l]
        lo, hi = LAYER_SL[l]
        rows = []
        for j in range(k):
            rows.append(value[b, lo:hi][j::k])
            for s in range(3):
                rows.append(position[b, lo:hi, s][j::k])
        idx = np.stack(rows).astype(np.int64)
        out[f"r8_{name}"], out[f"rq_{name}"] = _pack_ridx(idx)
    return out


_CACHE = {}

# schedule tuning knobs
PAIR = 1  # chunks per eq op (mixed-engine routing requires 1)
BPS_BUFS = 5
OPS_BUFS = 3
MH_BUFS = 3
ACT_NUM, ACT_DEN = 3, 7  # fraction of one-hot builds routed to the ACT path
DEPTH = 3
STAGE = "full"  # "full" | "mh_only" | "main_only" (HW bisection)
TT_PAIR = 2  # main t-tiles packed per PSUM bank
SMALL_ORD = 1  # small-stripe placement in the big-stripe interleave


def _get_nc(layers, nchunks, scale, reps=1):
    key = ("v3", PAIR, BPS_BUFS, OPS_BUFS, MH_BUFS, ACT_NUM, ACT_DEN, DEPTH,
           reps, STAGE, TT_PAIR, SMALL_ORD, scale,
           tuple((n, t, o, tuple(c)) for n, t, o, c in layers))
    if key in _CACHE:
        return _CACHE[key]

    import concourse.bass as bass
    import concourse.tile as tile
    from concourse import bacc, mybir
    from contextlib import ExitStack

    f32 = mybir.dt.float32
    f32r = mybir.dt.float32r
    bf16 = mybir.dt.bfloat16
    fp8 = mybir.dt.float8e4

    nc = bacc.Bacc(trn_type="TRN2", target_bir_lowering=False, debug=False)
    tbl_d = nc.dram_tensor("tbl", [128, nchunks * 2 * E], fp8, kind="ExternalInput").ap()
    sel8_d = nc.dram_tensor(
        "sel8", [NIDX, nchunks * 256], fp8, kind="ExternalInput"
    ).ap()
    selq_d = nc.dram_tensor(
        "selq", [NIDXQ, nchunks * 128], bf16, kind="ExternalInput"
    ).ap()
    r8_d = {
        name: nc.dram_tensor(f"r8_{name}", [NIDX, 2, T], fp8, kind="ExternalInput").ap()
        for name, T, _, _ in layers
    }
    rq_d = {
        name: nc.dram_tensor(f"rq_{name}", [NIDXQ, T], bf16, kind="ExternalInput").ap()
        for name, T, _, _ in layers
    }
    out_d = nc.dram_tensor("out", [OUT_TOKENS, E], f32, kind="ExternalOutput").ap()

    with tile.TileContext(nc) as tc, ExitStack() as ctx:
        cpool = ctx.enter_context(tc.tile_pool(name="const", bufs=1))
        rpool = ctx.enter_context(tc.tile_pool(name="ridx", bufs=DEPTH + 1))
        mpool = ctx.enter_context(tc.tile_pool(name="mh", bufs=MH_BUFS))
        tpool = ctx.enter_context(tc.tile_pool(name="sq", bufs=3))
        bps = ctx.enter_context(
            tc.tile_pool(name="bps", bufs=BPS_BUFS, space=bass.MemorySpace.PSUM)
        )
        ops = ctx.enter_context(
            tc.tile_pool(name="ops", bufs=OPS_BUFS, space=bass.MemorySpace.PSUM)
        )
        opool = ctx.enter_context(tc.tile_pool(name="osb", bufs=3))

        A = mybir.ActivationFunctionType
        stripes = []
        for name, T, out_off, cids in layers:
            for s0 in range(0, T, STRIPE):
                stripes.append((name, out_off, cids, s0, min(STRIPE, T - s0)))
        # spread the small stripes (B/L3) between the big L4 ones
        big = [s for s in stripes if s[0] == "L4"]
        small = [s for s in stripes if s[0] != "L4"]
        if SMALL_ORD == 0:
            small.sort(key=lambda s: -s[4])  # tiny tail stripe goes last
        elif SMALL_ORD == 1:
            small.sort(key=lambda s: s[4])  # tiny stripes early, big L4 last
        stripes = []
        for i, b in enumerate(big):
            stripes.append(b)
            if i * len(small) // len(big) < (i + 1) * len(small) // len(big):
                stripes.append(small[i * len(small) // len(big)])
        if SMALL_ORD == 2:
            # all smalls up front after the first big stripe
            stripes = [big[0]] + small + big[1:]

        def load_r8(si):
            name, _, cids, s0, W = stripes[si]
            rt8 = rpool.tile([NIDX, 2, W], fp8, tag="r8")
            nc.sync.dma_start(rt8[:], r8_d[name][:, :, s0 : s0 + W])
            return rt8

        def load_rq(si):
            name, _, cids, s0, W = stripes[si]
            rtq = rpool.tile([NIDXQ, W], bf16, tag="rq")
            nc.sync.dma_start(rtq[:], rq_d[name][:, s0 : s0 + W])
            return rtq

        def load_ridx(si):
            return load_r8(si), load_rq(si)

        # DMA order: small sel8 first, then stripe 0's fp8 index rows — the
        # first (DVE-routed) broadcast matmuls start ~2us in — then the bulky
        # selq behind them, the remaining head ridx, and the table last.
        sel8_t = cpool.tile([NIDX, nchunks * 256], fp8, tag="sel8")
        nc.sync.dma_start(sel8_t[:], sel8_d[:])
        pre8_0 = load_r8(0)
        selq_t = cpool.tile([NIDXQ, nchunks * 128], bf16, tag="selq")
        nc.sync.dma_start(selq_t[:], selq_d[:])
        preload_rt = {0: (pre8_0, load_rq(0))}
        for si in range(1, min(DEPTH, len(stripes))):
            preload_rt[si] = load_ridx(si)
        tbl_t = cpool.tile([128, nchunks * 2 * E], fp8, tag="tbl")
        use_order = []
        for s in stripes:
            if s[0] not in use_order:
                use_order.append(s[0])
        lay_by_name = {name: cids for name, _, _, cids in layers}
        for name in use_order:
            cids = lay_by_name[name]
            lo, hi = cids[0] * 2 * E, (cids[-1] + 1) * 2 * E
            nc.sync.dma_start(tbl_t[:, lo:hi], tbl_d[:, lo:hi])

        route_ctr = [0]

        def build_mh_pair(si, rts, p, ks):
            """broadcast matmul + one-hot for one chunk instance, routed to
            one of two engine paths:
              DVE: fp8 DoubleRow bcast emits d (0.5 PE cyc/col), is_equal(.,0)
              ACT: bf16 bcast emits d^2 exactly, single-pass relu(1 - d^2)
            """
            _, _, cids, _, W = stripes[si]
            rt8, rtq = rts
            n = len(ks)
            g = route_ctr[0]
            route_ctr[0] += 1
            # first few builds forced to DVE: the ACT path's selq tensor is
            # the bulkiest constant and lands a little later
            to_act = g >= 6 and (g * ACT_NUM) % ACT_DEN < ACT_NUM
            bp = bps.tile([128, n * W], f32, tag="b")
            mh = mpool.tile([128, n * W], fp8, tag=f"mh{p}")
            # matmuls split per 512-col PSUM bank; the one-hot op spans all
            # banks in one pass to amortize the engine's PSUM access latency
            for i, k in enumerate(ks):
                ci = cids[k]
                for o in range(0, W, 512):
                    w = min(512, W - o)
                    if to_act:
                        nc.tensor.matmul(
                            bp[:, i * W + o : i * W + o + w],
                            selq_t[:, ci * 128 : (ci + 1) * 128],
                            rtq[:, o : o + w],
                            start=True,
                            stop=True,
                        )
                    else:
                        nc.tensor.matmul(
                            bp[:, i * W + o : i * W + o + w],
                            sel8_t[:, ci * 256 : (ci + 1) * 256].rearrange(
                                "p (t m) -> p t m", t=2
                            ),
                            rt8[:, :, o : o + w],
                            start=True,
                            stop=True,
                            perf_mode=mybir.MatmulPerfMode.DoubleRow,
                        )
            if to_act:
                nc.scalar.activation(mh[:], bp[:], A.Relu, bias=1.0, scale=-1.0)
            else:
                nc.vector.tensor_scalar(
                    mh[:], bp[:], 0.0, None, op0=mybir.AluOpType.is_equal
                )
            return [mh[:, i * W : (i + 1) * W] for i in range(n)]

        def main_ttile(si, mhs, ti, ob):
            """two t-tiles packed into one PSUM bank; one evict per pair."""
            _, _, cids, _, W = stripes[si]
            nt = min(TT_PAIR, -(-W // 128) - TT_PAIR * ti)
            op = ops.tile([128, nt * E], f32, tag="o")
            Ms = []
            for h in range(nt):
                t0 = (TT_PAIR * ti + h) * 128
                M = min(128, W - t0)
                Ms.append(M)
                for k, ci in enumerate(cids):
                    nc.tensor.matmul(
                        op[:M, h * E : h * E + E],
                        mhs[k][:, t0 : t0 + M].unsqueeze(1).broadcast_to([128, 2, M]),
                        tbl_t[:, ci * 2 * E : (ci + 1) * 2 * E].rearrange(
                            "p (t e) -> p t e", t=2
                        ),
                        start=(k == 0),
                        stop=(k == len(cids) - 1),
                        perf_mode=mybir.MatmulPerfMode.DoubleRow,
                    )
            col = TT_PAIR * ti * E
            dsc = 1.0 / scale
            if nt == 2 and Ms[0] == 128 and Ms[1] == 128:
                nc.scalar.activation(ob[:, col : col + 2 * E], op[:], A.Copy, scale=dsc)
            else:
                for h in range(nt):
                    nc.scalar.activation(
                        ob[: Ms[h], col + h * E : col + (h + 1) * E],
                        op[: Ms[h], h * E : h * E + E],
                        A.Copy,
                        scale=dsc,
                    )

        def store_tt(si, ob, ti):
            """stream each t-tile group out right after its PSUM evict, so the
            final stripe doesn't serialize a whole-stripe store at the end."""
            _, out_off, _, s0, W = stripes[si]
            ntt = -(-W // 128)
            nt = min(TT_PAIR, ntt - TT_PAIR * ti)
            t0 = TT_PAIR * ti * 128
            M = min(TT_PAIR * 128, W - t0)
            row = out_off + s0 + t0
            col = TT_PAIR * ti * E
            if M == nt * 128:
                dst = out_d[row : row + M, :].rearrange("(a p) e -> p a e", p=128)
                src = ob[:, col : col + nt * E].rearrange("p (a e) -> p a e", e=E)
                nc.sync.dma_start(dst, src)
            else:
                for h in range(nt):
                    th = t0 + h * 128
                    m = min(128, W - th)
                    nc.sync.dma_start(
                        out_d[row + h * 128 : row + h * 128 + m, :],
                        ob[:m, col + h * E : col + (h + 1) * E],
                    )

        # two-stripe software pipeline with interleaved emission: pair-builds
        # of stripe s+2's one-hots alternate with stripe s's main t-tiles.
        def stripe_pairs(si):
            nk = len(stripes[si][2])
            return [tuple(range(a, min(a + PAIR, nk))) for a in range(0, nk, PAIR)]

        def emit_pairs(si, rt, prs):
            mhs = []
            for p, ks in prs:
                mhs += build_mh_pair(si, rt, p, ks)
            return mhs

        def emit_body_mh_only():
            for si in range(len(stripes)):
                rt = load_ridx(si)
                emit_pairs(si, rt, list(enumerate(stripe_pairs(si))))

        static_mh = {}
        if STAGE == "main_only":
            tmp0 = cpool.tile([128, STRIPE], f32, tag="smhtmp")
            nc.gpsimd.memset(tmp0[:], 0.5)
            for p in range(13):
                t = cpool.tile([128, STRIPE], fp8, tag=f"smh{p}")
                nc.vector.tensor_scalar(
                    t[:], tmp0[:], 0.0, None, op0=mybir.AluOpType.is_equal
                )
                static_mh[p] = t

        def emit_body_main_only():
            for si in range(len(stripes)):
                _, _, cids, _, W = stripes[si]
                ntt = -(-W // 128)
                ob = opool.tile([128, ntt * E], f32, tag="ob")
                mhs = [static_mh[k][:, :W] for k in range(len(cids))]
                for ti in range(-(-ntt // TT_PAIR)):
                    main_ttile(si, mhs, ti, ob)
                    store_tt(si, ob, ti)

        def emit_body():
            nst = len(stripes)
            mh_of = {}
            for si in range(min(DEPTH, nst)):
                mh_of[si] = emit_pairs(
                    si, preload_rt[si], list(enumerate(stripe_pairs(si)))
                )
            for si in range(nst):
                W = stripes[si][4]
                ntt = -(-W // 128)
                ngr = -(-ntt // TT_PAIR)
                ob = opool.tile([128, ntt * E], f32, tag="ob")
                sj = si + DEPTH
                if sj < nst:
                    rt = load_ridx(sj)
                    prs = list(enumerate(stripe_pairs(sj)))
                    npr = len(prs)
                    # split stripe sj's pair-builds into groups interleaved
                    # with stripe si's main t-tile pairs
                    bounds = [round(g * npr / ngr) for g in range(ngr + 1)]
                    mh_of[sj] = []
                    for ti in range(ngr):
                        main_ttile(si, mh_of[si], ti, ob)
                        store_tt(si, ob, ti)
                        mh_of[sj] += emit_pairs(
                            sj, rt, prs[bounds[ti] : bounds[ti + 1]]
                        )
                else:
                    for ti in range(ngr):
                        main_ttile(si, mh_of[si], ti, ob)
                        store_tt(si, ob, ti)
                del mh_of[si]

        body_fn = {
            "full": emit_body,
            "mh_only": emit_body_mh_only,
            "main_only": emit_body_main_only,
        }[STAGE]
        if reps == 1:
            body_fn()
        else:
            # timing mode: repeat the body on-device to measure per-iter HW
            # time as a wall-clock slope (no NTFF profiling available)
            hints = (
                mybir.EngineType.PE,
                mybir.EngineType.DVE,
                mybir.EngineType.Activation,
                mybir.EngineType.SP,
            )
            with tc.For_i(0, reps, 1, hint_engines=hints):
                body_fn()

    nc.compile()
    _CACHE[key] = nc
    return nc


def kernel(**inputs):
    from concourse.bass_utils import run_bass_kernel_spmd

    value = np.asarray(inputs["value"], np.int32)
    depth = np.asarray(inputs["depth"], np.int32)
    position = np.asarray(inputs["position"], np.int32)

    tbl, scale, sel8, selq, layers = _build_consts(inputs)
    nc = _get_nc(layers, tbl.shape[1] // (2 * E), scale)

    in_maps = []
    for b in range(BATCH):
        rid = _build_ridx(value, depth, position, b)
        m = {"tbl": tbl, "sel8": sel8, "selq": selq}
        for name, _, _, _ in layers:
            m[f"r8_{name}"] = rid[f"r8_{name}"]
            m[f"rq_{name}"] = rid[f"rq_{name}"]
        in_maps.append(m)

    res = run_bass_kernel_spmd(nc, in_maps, list(range(BATCH)))
    return np.stack([res.results[b]["out"] for b in range(BATCH)])

